# revision 1
# baseline (speedup 1.0000x reference)
"""Trainium2 Bass kernel for the sliding-window (sparse block) attention layer.

Problem shape: B=1, C=2048, L=16384, projected c=1024, block bl=512, nb=32
blocks, window 2*bl=1024 with halo bl//2=256.

Sharding: sequence-parallel over the nb block dimension; each of the 8 cores
owns 4 consecutive blocks (2048 columns).  The k/v halo (bl//2 = 256 columns
each side) is exchanged between sequence-neighbour cores with pairwise
AllGather collectives instead of being recomputed locally, removing ~9% of
the per-core matmul work.

Halo exchange topology: the runtime only supports replica groups aligned
with the mesh address bits (pairs differing in exactly one bit), so shards
are laid onto cores in GRAY-CODE order (shard s -> core [0,1,3,2,6,7,5,4][s]).
Then every sequence boundary is a one-bit pair, covered by one of three
axis AllGathers (bit0: (0,1)(2,3)(4,5)(6,7); bit1: (0,2)(1,3)(4,6)(5,7);
bit2: (0,4)(1,5)(2,6)(3,7)).  Every core contributes the same symmetric
payload [kL|kR|vL|vR] (its own 256 edge columns in both the k and the vT
staging layouts) to all three.  An AllGather output is the linear
concatenation [lower rank's payload | higher rank's payload]; which gather
and which half hold a core's actual neighbour is a per-core constant, so
the program stays SPMD-uniform by blending all six candidates with one-hot
per-core selector weights (a host input).  The outermost shard edges use
all-zero selectors, reproducing the reference's zero padding exactly.

Per-core device pipeline (all matmuls in bf16, f32 PSUM accumulation):
  Phase 1 (single x1 pass per l-chunk): k = wk@x1s+bk as (c, chunk) and
           vT = (wv@x1s)^T computed directly in transposed layout; both
           staged to internal DRAM in bf16 at +256 slab offset (the halo
           region around them is filled by the blended exchange).  The two
           edge chunks run first so the collectives launch ~20% into
           phase 1; wk is staged ci-blocked (8 tiles), wv ch-blocked (2),
           and x1 lands in 16 per-contraction-chunk tiles so the first PE
           groups start as soon as their specific DMA lands.
  Phase 2 (per block, in order 1,2,0,3 so the halo-dependent blocks run
           last and the collective latency hides behind real work):
           qb = wq@x1b+bq; ST = kb^T qb (keys on partitions); P~T =
           exp(ST/sqrt(c) + logmask) via one ACT op; denom(l) via DVE
           running sum + one ones-matmul; recip broadcast to 128 partitions
           via a K=1 matmul (never touches GpSimd); out = vT^T @ P~T in
           PSUM, then x*recip+bv, relu; final = woT^T @ relu(..) + bo.

The reference's softmax(energy + log(mask+1e-9)) is reproduced exactly by
the additive log-mask; the post-softmax *mask multiply only zeroes
~1e-9-weight entries and is skipped (the final `out * mask` is host-side).

Scheduling: DRAM reads ride the SP HWDGE queue, DRAM writes the SWDGE
(gpsimd) queue; the ACT sequencer never issues bulk DMA (kicks there delay
PSUM evacuations and stall the PE).  Phase-2 inputs (wq, wo, first x1
block, first k/v window halves) prefetch from inside phase 1.
"""

import os
import sys

import numpy as np

for _p in ("/root/.axon_site", "/root/.axon_site/_ro/trn_rl_repo", "/opt/trn_rl_repo"):
    if os.path.isdir(_p) and _p not in sys.path:
        sys.path.append(_p)

import ml_dtypes

import concourse.bass as bass
import concourse.mybir as mybir
import concourse.tile as tile
from concourse import bacc
from concourse.bass import ds, ts

BF16 = ml_dtypes.bfloat16

# Model dims (hardcoded per problem spec)
C = 2048          # input channels
CQ = 1024         # projected channels
L = 16384         # sequence length
BL = 512          # block length
HALF = 256        # halo = BL // 2
NCORES = 8
LCORE = L // NCORES          # 2048 owned columns per core
LH = LCORE + 2 * HALF        # 2560 slab columns incl halo
NBLK = LCORE // BL           # 4 blocks per core
WIN = 2 * BL                 # 1024 attention window
ESCALE = 1.0 / float(np.sqrt(CQ))  # 1/32

NCI = C // 128    # 16 contraction chunks over C
NCQ = CQ // 128   # 8 chunks over projected c
NCO = C // 128    # 16 chunks over output channels
NMC = WIN // 128  # 8 key chunks per window

# phase-1 l-chunk grid over the OWN 2048 columns; the two edge chunks run
# first so the halo exchange can start early, and they double as the
# fast-start split (smaller first matmul groups).
LCHUNKS = [(0, 256), (1792, 256), (256, 256), (512, 512), (1024, 512),
           (1536, 256)]

# shard s lives on core GRAY[s]; consecutive shards differ in one bit
GRAY = [0, 1, 3, 2, 6, 7, 5, 4]
AGS = [
    [[0, 1], [2, 3], [4, 5], [6, 7]],   # bit 0
    [[0, 2], [1, 3], [4, 6], [5, 7]],   # bit 1
    [[0, 4], [1, 5], [2, 6], [3, 7]],   # bit 2
]
# phase-2 block order: halo-free blocks first
BORDER = [1, 2, 0, 3]


def build_kernel() -> bass.Bass:
    nc = bacc.Bacc("TRN2", target_bir_lowering=False, num_devices=NCORES)
    dt = mybir.dt
    f32, bf16 = dt.float32, dt.bfloat16
    AFT = mybir.ActivationFunctionType

    x1s = nc.dram_tensor("x1s", [C, LCORE], bf16, kind="ExternalInput")
    wkB = nc.dram_tensor("wkB", [128, NCQ, NCI, 128], bf16, kind="ExternalInput")
    wvB = nc.dram_tensor("wvB", [128, 2, NCI, BL], bf16, kind="ExternalInput")
    wqT = nc.dram_tensor("wqT", [C, CQ], bf16, kind="ExternalInput")
    woT = nc.dram_tensor("woT", [CQ, C], bf16, kind="ExternalInput")
    bq = nc.dram_tensor("bq", [128, CQ // 128], f32, kind="ExternalInput")
    bk = nc.dram_tensor("bk", [128, CQ // 128], f32, kind="ExternalInput")
    bv = nc.dram_tensor("bv", [128, CQ // 128], f32, kind="ExternalInput")
    bo = nc.dram_tensor("bo", [128, C // 128], f32, kind="ExternalInput")
    amask = nc.dram_tensor("amask", [128, NBLK * (WIN // 128)], f32,
                           kind="ExternalInput")
    selv = nc.dram_tensor("selv", [128, 12], f32, kind="ExternalInput")
    out = nc.dram_tensor("out", [C, LCORE], f32, kind="ExternalOutput")

    # Internal DRAM staging for k (c-major) and vT (m-major), in SLAB
    # coordinates: own data at +256 cols (+2 rows), halo around it.
    kst = nc.dram_tensor("kst", [128, CQ // 128, LH], bf16)
    vst = nc.dram_tensor("vst", [128, LH // 128, CQ], bf16)

    # halo exchange buffers: payload [kL|kR|vL|vR], 2048 bf16 per slot/part
    send = nc.dram_tensor("send", [128, 4, 2048], bf16)
    sk = send.rearrange("p s (b c) -> p s b c", b=NCQ)     # (128,4,8,256)
    sv = send.rearrange("p s (b c) -> p s b c", b=2)       # (128,4,2,1024)
    recvs = []
    rks, rvs = [], []
    for i in range(3):
        rc = nc.dram_tensor(f"recv{i}", [2, 128, 4, 2048], bf16)
        recvs.append(rc)
        rks.append(rc.rearrange("g p s (b c) -> g p s b c", b=NCQ))
        rvs.append(rc.rearrange("g p s (b c) -> g p s b c", b=2))

    x1r = x1s.rearrange("(ci p) l -> p ci l", p=128)    # (128, 16, 2048)
    wqr = wqT.rearrange("(ci p) c -> p ci c", p=128)    # (128, 16, 1024)
    wor = woT.rearrange("(ci p) co -> p ci co", p=128)  # (128, 8, 2048)
    outr = out.rearrange("(co p) l -> p co l", p=128)   # (128, 16, 2048)

    with tile.TileContext(nc) as tc:
        with (
            tc.tile_pool(name="singles", bufs=1) as singles,
            tc.tile_pool(name="x1pool", bufs=2) as x1pool,
            tc.tile_pool(name="kroll", bufs=3) as krollp,
            tc.tile_pool(name="vroll", bufs=3) as vrollp,
            tc.tile_pool(name="wqo", bufs=1) as wqo,
        ):
            def load_x1_chunk(col0, ncols, split=False):
                """x1 chunk as 16 per-Ci tiles so each matmul only waits on
                the one DMA it actually reads.  split=True sends odd chunks
                down the ACT HWDGE queue (safe while ACT is still idle)."""
                tiles = []
                for Ci in range(NCI):
                    t = x1pool.tile([128, ncols], bf16, tag=f"x1c{Ci}",
                                    name=f"x1c{Ci}")
                    eng = nc.scalar if (split and Ci % 2) else nc.sync
                    eng.dma_start(t, x1r[:, Ci, ds(col0, ncols)])
                    tiles.append(t)
                return tiles

            khalves: dict[int, bass.AP] = {}
            vhalves: dict[int, bass.AP] = {}

            def load_half(h: int):
                kh = krollp.tile([128, NCQ, BL], bf16, tag="kh")
                for ci in range(NCQ):
                    nc.sync.dma_start(kh[:, ci], kst[:, ci, ts(h, BL)])
                vh = vrollp.tile([128, BL // 128, CQ], bf16, tag="vh")
                for mo in range(BL // 128):
                    nc.sync.dma_start(vh[:, mo], vst[:, h * 4 + mo, :])
                khalves[h] = kh
                vhalves[h] = vh

            sel_sb = singles.tile([128, 12], f32)
            nc.gpsimd.dma_start(sel_sb, selv[:, :])

            def blend(dst_dma, cands, scol, shape):
                """dst = sum_i cands[i]*sel[scol+i] (one-hot or all-zero
                selector); one whole halo region per unit so the six
                candidate DMAs pipeline instead of serialising."""
                acc = halop.tile(shape, bf16, tag="acc", name="acc")
                for i, cnd in enumerate(cands):
                    cb = halop.tile(shape, bf16, tag="cand", name="cand", bufs=3)
                    nc.gpsimd.dma_start(cb, cnd)
                    if i == 0:
                        nc.vector.tensor_scalar_mul(
                            acc, cb, sel_sb[:, scol:scol + 1])
                    else:
                        nc.vector.tensor_scalar_mul(
                            cb, cb, sel_sb[:, scol + i:scol + i + 1])
                        nc.vector.tensor_add(acc, acc, cb)
                dst_dma(acc)

            def emit_blends():
                # left halo source slot: kR(1)/vR(3); right: kL(0)/vL(2).
                # candidate order: (axis0 LO, axis0 HI, axis1 LO, ... )
                blend(lambda hb: nc.gpsimd.dma_start(
                          kst[:, :, ds(0, HALF)], hb),
                      [rks[i][h, :, 1] for i in range(3) for h in range(2)],
                      0, [128, NCQ, HALF])
                blend(lambda hb: nc.gpsimd.dma_start(
                          kst[:, :, ds(LH - HALF, HALF)], hb),
                      [rks[i][h, :, 0] for i in range(3) for h in range(2)],
                      6, [128, NCQ, HALF])
                blend(lambda hb: nc.gpsimd.dma_start(
                          vst[:, 0:2, :], hb),
                      [rvs[i][h, :, 3] for i in range(3) for h in range(2)],
                      0, [128, 2, CQ])
                blend(lambda hb: nc.gpsimd.dma_start(
                          vst[:, LH // 128 - 2:LH // 128, :], hb),
                      [rvs[i][h, :, 2] for i in range(3) for h in range(2)],
                      6, [128, 2, CQ])

            # ---------------- Phase 1: k and vT -> DRAM ----------------
            with (
                tc.tile_pool(name="wkv", bufs=1) as wkv,
                tc.tile_pool(name="stage", bufs=3) as stage,
                tc.tile_pool(name="psum1", bufs=3, space="PSUM") as psum1,
            ):
                wk_t = []
                for ci in range(NCQ):
                    wk_t.append(wkv.tile([128, NCI, 128], bf16, tag=f"wk{ci}",
                                         name=f"wk{ci}"))
                wv_t = []
                for ch in range(2):
                    wv_t.append(wkv.tile([128, NCI, BL], bf16, tag=f"wv{ch}",
                                         name=f"wv{ch}"))

                nc.sync.dma_start(wk_t[0], wkB[:, 0])
                x1a = load_x1_chunk(*LCHUNKS[0], split=True)
                for ci in range(1, NCQ):
                    nc.sync.dma_start(wk_t[ci], wkB[:, ci])
                x1b_first = load_x1_chunk(*LCHUNKS[1], split=True)
                for ch in range(2):
                    nc.sync.dma_start(wv_t[ch], wvB[:, ch])

                bq_sb = singles.tile([128, NCQ], f32)
                nc.gpsimd.dma_start(bq_sb, bq[:, :])
                bk_sb = singles.tile([128, NCQ], f32)
                nc.gpsimd.dma_start(bk_sb, bk[:, :])
                bv_sb = singles.tile([128, NCQ], f32)
                nc.gpsimd.dma_start(bv_sb, bv[:, :])
                bo_sb = singles.tile([128, NCO], f32)
                nc.gpsimd.dma_start(bo_sb, bo[:, :])
                am_sb = singles.tile([128, NBLK * NMC], f32)
                nc.gpsimd.dma_start(am_sb, amask[:, :])
                onesc_bf = singles.tile([128, 1], bf16)
                nc.vector.memset(onesc_bf, 1.0)
                onesr_bf = singles.tile([1, 128], bf16)
                nc.vector.memset(onesr_bf, 1.0)

                def emit_k(x1t, col0, ncols, edge=None):
                    for ci in range(NCQ):
                        ps = psum1.tile([128, ncols], f32, tag="pk")
                        for Ci in range(NCI):
                            nc.tensor.matmul(
                                ps,
                                lhsT=wk_t[ci][:, Ci],
                                rhs=x1t[Ci],
                                start=(Ci == 0),
                                stop=(Ci == NCI - 1),
                            )
                        kt = stage.tile([128, ncols], bf16, tag="kst")
                        nc.scalar.add(kt, ps, bk_sb[:, ci:ci + 1])
                        nc.scalar.dma_start(
                            kst[:, ci, ds(col0 + HALF, ncols)], kt)
                        if edge is not None:
                            slot = 0 if edge == "L" else 1
                            nc.scalar.dma_start(sk[:, slot, ci], kt)

                def emit_v(x1t, col0, ncols, edge=None):
                    for mo in range(ncols // 128):
                        for ch in range(2):
                            ps = psum1.tile([128, BL], f32, tag="pv")
                            for Ci in range(NCI):
                                nc.tensor.matmul(
                                    ps,
                                    lhsT=x1t[Ci][:, ts(mo, 128)],
                                    rhs=wv_t[ch][:, Ci],
                                    start=(Ci == 0),
                                    stop=(Ci == NCI - 1),
                                )
                            vt = stage.tile([128, BL], bf16, tag="vst")
                            nc.vector.tensor_copy(vt, ps)
                            nc.sync.dma_start(
                                vst[:, 2 + col0 // 128 + mo, ts(ch, BL)], vt)
                            if edge is not None:
                                slot = 2 if edge == "L" else 3
                                nc.sync.dma_start(
                                    sv[:, slot, mo, ts(ch, BL)], vt)

                wq_sb = wqo.tile([128, NCI, CQ], bf16, tag="wq")
                x1b_pre = None
                emit_k(x1a, *LCHUNKS[0], edge="L")
                emit_k(x1b_first, *LCHUNKS[1], edge="R")
                emit_v(x1a, *LCHUNKS[0], edge="L")
                emit_v(x1b_first, *LCHUNKS[1], edge="R")
                # both edges staged -> launch the halo exchange
                for i in range(3):
                    nc.gpsimd.collective_compute(
                        "AllGather", mybir.AluOpType.bypass,
                        replica_groups=AGS[i],
                        ins=[send[:, :, :]], outs=[recvs[i][:, :, :, :]])
                for li in range(2, len(LCHUNKS)):
                    col0, ncols = LCHUNKS[li]
                    x1t = load_x1_chunk(col0, ncols)
                    emit_k(x1t, col0, ncols)
                    emit_v(x1t, col0, ncols)
                    if li == 2:
                        for Ci in range(NCI):
                            nc.sync.dma_start(wq_sb[:, Ci], wqr[:, Ci, :])
                    elif li == 3:
                        load_half(1)
                    elif li == 4:
                        load_half(2)
                    elif li == len(LCHUNKS) - 1:
                        x1b_pre = load_x1_chunk(BORDER[0] * BL, BL)

            # ---------------- Phase 2: attention + output proj ----------------
            with (
                tc.tile_pool(name="wop", bufs=1) as wop,
                tc.tile_pool(name="halop", bufs=1) as halop,
                tc.tile_pool(name="qbp", bufs=1) as qbp,
                tc.tile_pool(name="ptp", bufs=8) as ptp,
                tc.tile_pool(name="small2", bufs=1) as small2,
                tc.tile_pool(name="relup", bufs=1) as relup,
                tc.tile_pool(name="osbp", bufs=3) as osbp,
                tc.tile_pool(name="tmpp", bufs=2) as tmpp,
                tc.tile_pool(name="dacc", bufs=1) as daccp,
                tc.tile_pool(name="psum2", bufs=2, space="PSUM") as psum2,
                tc.tile_pool(name="psumav", bufs=3, space="PSUM") as psumav,
                tc.tile_pool(name="psumd", bufs=1, space="PSUM") as psumd,
            ):
                wo_sb = wop.tile([128, NCQ, C], bf16, tag="wo")
                for ci in range(NCQ):
                    nc.sync.dma_start(wo_sb[:, ci], wor[:, ci, :])
                for bi, b in enumerate(BORDER):
                    if bi == 0:
                        x1b = x1b_pre
                    else:
                        x1b = load_x1_chunk(b * BL, BL)
                        for h in (b, b + 1):
                            if h not in khalves:
                                load_half(h)

                    # q projection for this block
                    qb_sb = qbp.tile([128, NCQ, BL], bf16, tag="qb")
                    for ci in range(NCQ):
                        ps = psum2.tile([128, BL], f32, tag="st")
                        for Ci in range(NCI):
                            nc.tensor.matmul(
                                ps,
                                lhsT=wq_sb[:, Ci, ts(ci, 128)],
                                rhs=x1b[Ci],
                                start=(Ci == 0),
                                stop=(Ci == NCI - 1),
                            )
                        nc.scalar.add(qb_sb[:, ci], ps, bq_sb[:, ci:ci + 1])

                    # energy^T tiles (keys on partitions) + exp; denominator
                    # partials accumulate on the (idle) DVE
                    acc = daccp.tile([128, BL], f32, tag="acc")
                    pts = []
                    for mc in range(NMC):
                        kh = khalves[b + mc // 4]
                        off = (mc % 4) * 128
                        ps_st = psum2.tile([128, BL], f32, tag="st")
                        for ci in range(NCQ):
                            nc.tensor.matmul(
                                ps_st,
                                lhsT=kh[:, ci, ds(off, 128)],
                                rhs=qb_sb[:, ci, :],
                                start=(ci == 0),
                                stop=(ci == NCQ - 1),
                            )
                        pt = ptp.tile([128, BL], bf16, tag="pt")
                        col = b * NMC + mc
                        nc.scalar.activation(
                            pt, ps_st, AFT.Exp,
                            bias=am_sb[:, col:col + 1], scale=ESCALE)
                        pts.append(pt)
                        if mc == 0:
                            nc.vector.tensor_copy(acc, pt)
                        else:
                            nc.vector.tensor_add(acc, acc, pt)

                    # denom -> recip -> partition-broadcast, PE+DVE only
                    accb = daccp.tile([128, BL], bf16, tag="accb")
                    nc.vector.tensor_copy(accb, acc)
                    ps_den = psumd.tile([128, BL], f32, tag="den")
                    nc.tensor.matmul(
                        ps_den[0:1, :], lhsT=onesc_bf, rhs=accb,
                        start=True, stop=True)
                    recip = small2.tile([1, BL], f32, tag="recip")
                    nc.vector.reciprocal_approx_fast(recip, ps_den[0:1, :])
                    recip16 = small2.tile([1, BL], bf16, tag="recip16")
                    nc.vector.tensor_copy(recip16, recip)
                    ps_denb = psumd.tile([128, BL], f32, tag="den")
                    nc.tensor.matmul(
                        ps_denb, lhsT=onesr_bf, rhs=recip16,
                        start=True, stop=True)
                    recipb = small2.tile([128, BL], f32, tag="recipb")
                    nc.vector.tensor_copy(recipb, ps_denb)

                    # attention * V, divide by denom, +bv, relu
                    relu_b = relup.tile([128, NCQ, BL], bf16, tag="relu")
                    for ci in range(NCQ):
                        ps_av = psumav.tile([128, BL], f32, tag="av")
                        for mc in range(NMC):
                            vh = vhalves[b + mc // 4]
                            nc.tensor.matmul(
                                ps_av,
                                lhsT=vh[:, mc % 4, ts(ci, 128)],
                                rhs=pts[mc],
                                start=(mc == 0),
                                stop=(mc == NMC - 1),
                            )
                        tmp = tmpp.tile([128, BL], f32, tag="tmp")
                        nc.vector.tensor_mul(tmp, ps_av, recipb)
                        nc.scalar.activation(
                            relu_b[:, ci], tmp, AFT.Relu,
                            bias=bv_sb[:, ci:ci + 1], scale=1.0)

                    # output projection
                    for co in range(NCO):
                        ps_o = psum2.tile([128, BL], f32, tag="proj")
                        for ci in range(NCQ):
                            nc.tensor.matmul(
                                ps_o,
                                lhsT=wo_sb[:, ci, ts(co, 128)],
                                rhs=relu_b[:, ci, :],
                                start=(ci == 0),
                                stop=(ci == NCQ - 1),
                            )
                        osb = osbp.tile([128, BL], f32, tag="osb")
                        nc.scalar.add(osb, ps_o, bo_sb[:, co:co + 1])
                        nc.scalar.dma_start(outr[:, co, ts(b, BL)], osb)

                    if bi == 1:
                        # blends (which wait on the collectives) enter the
                        # engine/DMA FIFOs only after block 2's traffic, so
                        # a slow collective can only ever delay blocks 0/3 -
                        # the two that consume the halo anyway
                        emit_blends()

    nc.finalize()
    return nc


def _part_major(v: np.ndarray) -> np.ndarray:
    """(n*128,) f32 vector -> (128, n) partition-major layout."""
    return np.ascontiguousarray(v.reshape(-1, 128).T).astype(np.float32)


def make_in_maps(x1, mask, wq, bq, wk, bk, wv, bv, wo, bo):
    X = np.asarray(x1[0], dtype=np.float32).astype(BF16)  # (C, L)

    wqT = np.ascontiguousarray(np.asarray(wq, np.float32).T).astype(BF16)
    wkT = np.ascontiguousarray(np.asarray(wk, np.float32).T).astype(BF16)
    wvT = np.ascontiguousarray(np.asarray(wv, np.float32).T).astype(BF16)
    woT = np.ascontiguousarray(np.asarray(wo, np.float32).T).astype(BF16)
    wkBd = np.ascontiguousarray(
        wkT.reshape(NCI, 128, NCQ, 128).transpose(1, 2, 0, 3))
    wvBd = np.ascontiguousarray(
        wvT.reshape(NCI, 128, 2, BL).transpose(1, 2, 0, 3))
    bqd = _part_major(np.asarray(bq, np.float32))
    bkd = _part_major(np.asarray(bk, np.float32))
    bvd = _part_major(np.asarray(bv, np.float32))
    bod = _part_major(np.asarray(bo, np.float32))

    # additive log-mask per global block: log(window_mask * padded_mask + 1e-9)
    pmpad = np.zeros(L + 2 * HALF, np.float32)
    pmpad[HALF:HALF + L] = np.asarray(mask, np.float32)[0, 0]
    wmcol = np.ones(WIN, np.float32)
    wmcol[-1] = 0.0
    nb_glob = L // BL
    fm = np.stack([wmcol * pmpad[bg * BL: bg * BL + WIN]
                   for bg in range(nb_glob)])  # (32, 1024)
    am_all = np.log(fm + 1e-9).astype(np.float32)

    axis_of = {1: 0, 2: 1, 4: 2}

    in_maps = [None] * NCORES
    for s in range(NCORES):       # shard index
        core = GRAY[s]
        x1sl = np.ascontiguousarray(X[:, s * LCORE:(s + 1) * LCORE])
        amc = am_all[s * NBLK:(s + 1) * NBLK]                # (4, 1024)
        amd = amc.reshape(NBLK, WIN // 128, 128).transpose(2, 0, 1)
        amd = np.ascontiguousarray(amd.reshape(128, NBLK * (WIN // 128)))
        # selectors: [left 6][right 6], entry (axis*2 + half)
        sel = np.zeros(12, np.float32)
        if s > 0:
            xl = GRAY[s - 1]
            ax = axis_of[xl ^ core]
            sel[ax * 2 + (1 if xl > core else 0)] = 1.0
        if s < NCORES - 1:
            xr = GRAY[s + 1]
            ax = axis_of[xr ^ core]
            sel[6 + ax * 2 + (1 if xr > core else 0)] = 1.0
        selc = np.ascontiguousarray(np.tile(sel, (128, 1)))
        in_maps[core] = {
            "x1s": x1sl, "wkB": wkBd, "wvB": wvBd, "wqT": wqT, "woT": woT,
            "bq": bqd, "bk": bkd, "bv": bvd, "bo": bod, "amask": amd,
            "selv": selc,
        }
    return in_maps


_CACHED = {}


def kernel(**inputs) -> np.ndarray:
    x1 = np.asarray(inputs["x1"])
    mask = np.asarray(inputs["mask"])
    in_maps = make_in_maps(
        x1, mask,
        inputs["wq"], inputs["bq"], inputs["wk"], inputs["bk"],
        inputs["wv"], inputs["bv"], inputs["wo"], inputs["bo"])

    from concourse.bass_utils import run_bass_kernel_spmd

    if "nc" not in _CACHED:
        _CACHED["nc"] = build_kernel()
    nc = _CACHED["nc"]

    res = run_bass_kernel_spmd(nc, in_maps, core_ids=list(range(NCORES)))
    # core GRAY[s] holds shard s
    outs = [np.asarray(res.results[GRAY[s]]["out"]) for s in range(NCORES)]
    full = np.concatenate(outs, axis=1)[None]          # (1, C, L)
    full = full * np.asarray(mask, np.float32)[:, 0:1, :]
    return np.ascontiguousarray(full.astype(np.float32))


if __name__ == "__main__":
    nc = build_kernel()
    print("built ok")



# revision 4
# speedup vs baseline: 1.0397x; 1.0397x over previous
"""Trainium2 Bass kernel for the sliding-window (sparse block) attention layer.

Problem shape: B=1, C=2048, L=16384, projected c=1024, block bl=512, nb=32
blocks, window 2*bl=1024 with halo bl//2=256.

Sharding: sequence-parallel over the nb block dimension; each of the 8 cores
owns 4 consecutive blocks (2048 columns).  The k/v halo (bl//2 = 256 columns
each side) is exchanged between sequence-neighbour cores with pairwise
AllGather collectives (GRAY-code core layout, one AllGather per mesh address
bit, one-hot per-core selector blending — see make_in_maps).

v3 design (all-SBUF, no DRAM staging):
  The previous version staged k and vT to internal DRAM and read them back
  in phase 2; the staging DMA writes rode the ACT HWDGE queue and stalled
  behind the (slow, ~90us each) pairwise AllGathers' SDMA traffic, idling
  the PE ~65us at the phase boundary.  Now k, vT and q live entirely in
  SBUF across phases (kslab 40KB + vslab 40KB + qslab 32KB per partition)
  and phase boundaries carry no DMA dependencies at all:

  Phase 1a (single x1 pass): per l-chunk (the two 256-col edge chunks
    first, then three 512-col interior chunks) compute vT = (wv@x1)^T
    directly in transposed layout (interleaved dual-psum groups sharing
    each x1 LDWEIGHTS) and k = wk@x1+bk, evacuating straight into the
    SBUF slabs.  After the edge chunks, the [kL|kR|vL|vR] edge payload is
    DMA'd to the DRAM send buffer and the three axis AllGathers launch on
    the gpsimd queue (nothing else uses that queue until the blends).
    x1 tiles are single-buffered per contraction chunk (Ci<8 double) and
    chased by the consuming matmuls; v runs before k in every chunk so the
    wv pool frees one chunk early and wq (reusing wv's address range)
    prefetches during the last k chunk.
  Phase 1b (second x1 pass): q = wq@x1+bq into the q slab.  The halo
    blends (6 one-hot-selected AllGather candidates per halo region) are
    emitted at the start of 1b and write the slab halo regions directly.
  Phase 2 (per block, order 1,2,0,3 so halo-dependent blocks run last):
    S^T = k^T q per 128-key chunk; P~^T = exp(S^T/sqrt(c)+logmask) via one
    ACT op (per-key log-mask rides the ACT bias); softmax denominator via
    DVE running sum + a ones-matmul + reciprocal + a broadcast ones-matmul,
    both interleaved between the first AV groups so the PE never waits;
    out = vT^T P~^T, *recip +bv, relu; final = wo^T relu + bo, DMA'd out
    on the SP queue.  wo loads into wq's slots (same pool tags) as the
    last q-projection groups release them.

The reference's softmax(energy + log(mask+1e-9)) is reproduced exactly by
the additive log-mask; the post-softmax *mask multiply only zeroes
~1e-9-weight entries and is skipped (the final `out * mask` is host-side).
"""

import os
import sys

import numpy as np

for _p in ("/root/.axon_site", "/root/.axon_site/_ro/trn_rl_repo", "/opt/trn_rl_repo"):
    if os.path.isdir(_p) and _p not in sys.path:
        sys.path.append(_p)

import ml_dtypes

import concourse.bass as bass
import concourse.mybir as mybir
import concourse.tile as tile
from concourse import bacc
from concourse.bass import ds, ts

BF16 = ml_dtypes.bfloat16

# Model dims (hardcoded per problem spec)
C = 2048          # input channels
CQ = 1024         # projected channels
L = 16384         # sequence length
BL = 512          # block length
HALF = 256        # halo = BL // 2
NCORES = 8
LCORE = L // NCORES          # 2048 owned columns per core
LH = LCORE + 2 * HALF        # 2560 slab columns incl halo
NBLK = LCORE // BL           # 4 blocks per core
WIN = 2 * BL                 # 1024 attention window
ESCALE = 1.0 / float(np.sqrt(CQ))  # 1/32

NCI = C // 128    # 16 contraction chunks over C
NCQ = CQ // 128   # 8 chunks over projected c
NCO = C // 128    # 16 chunks over output channels
NMC = WIN // 128  # 8 key chunks per window

# phase-1 l-chunk grid over the OWN 2048 columns; the two edge chunks run
# first so the halo exchange launches early.
LCHUNKS = [(0, 256), (1792, 256), (256, 512), (768, 512), (1280, 512)]

# shard s lives on core GRAY[s]; consecutive shards differ in one bit
GRAY = [0, 1, 3, 2, 6, 7, 5, 4]
AGS = [
    [[0, 1], [2, 3], [4, 5], [6, 7]],   # bit 0
    [[0, 2], [1, 3], [4, 6], [5, 7]],   # bit 1
    [[0, 4], [1, 5], [2, 6], [3, 7]],   # bit 2
]
# phase-2 block order: halo-free blocks first
BORDER = [1, 2, 0, 3]


def build_kernel() -> bass.Bass:
    nc = bacc.Bacc("TRN2", target_bir_lowering=False, num_devices=NCORES)
    dt = mybir.dt
    f32, bf16 = dt.float32, dt.bfloat16
    AFT = mybir.ActivationFunctionType

    x1s = nc.dram_tensor("x1s", [C, LCORE], bf16, kind="ExternalInput")
    wkB = nc.dram_tensor("wkB", [128, NCQ, NCI, 128], bf16, kind="ExternalInput")
    wqB = nc.dram_tensor("wqB", [128, NCQ, NCI, 128], bf16, kind="ExternalInput")
    wvB = nc.dram_tensor("wvB", [128, 2, NCI, BL], bf16, kind="ExternalInput")
    woT = nc.dram_tensor("woT", [CQ, C], bf16, kind="ExternalInput")
    bq = nc.dram_tensor("bq", [128, CQ // 128], f32, kind="ExternalInput")
    bk = nc.dram_tensor("bk", [128, CQ // 128], f32, kind="ExternalInput")
    bv = nc.dram_tensor("bv", [128, CQ // 128], f32, kind="ExternalInput")
    bo = nc.dram_tensor("bo", [128, C // 128], f32, kind="ExternalInput")
    amask = nc.dram_tensor("amask", [128, NBLK * (WIN // 128)], f32,
                           kind="ExternalInput")
    selv = nc.dram_tensor("selv", [128, 12], f32, kind="ExternalInput")
    out = nc.dram_tensor("out", [C, LCORE], f32, kind="ExternalOutput")

    # halo exchange buffers: payload [kL|kR|vL|vR], 2048 bf16 per slot/part
    send = nc.dram_tensor("send", [128, 4, 2048], bf16)
    sk = send.rearrange("p s (b c) -> p s b c", b=NCQ)     # (128,4,8,256)
    sv = send.rearrange("p s (b c) -> p s b c", b=2)       # (128,4,2,1024)
    recvs = []
    rks, rvs = [], []
    for i in range(3):
        rc = nc.dram_tensor(f"recv{i}", [2, 128, 4, 2048], bf16)
        recvs.append(rc)
        rks.append(rc.rearrange("g p s (b c) -> g p s b c", b=NCQ))
        rvs.append(rc.rearrange("g p s (b c) -> g p s b c", b=2))

    x1r = x1s.rearrange("(ci p) l -> p ci l", p=128)    # (128, 16, 2048)
    wor = woT.rearrange("(ci p) co -> p ci co", p=128)  # (128, 8, 2048)
    outr = out.rearrange("(co p) l -> p co l", p=128)   # (128, 16, 2048)

    with tile.TileContext(nc) as tc:
        with (
            tc.tile_pool(name="singles", bufs=1) as singles,
            tc.tile_pool(name="kslabp", bufs=1) as kslabp,
            tc.tile_pool(name="vslabp", bufs=1) as vslabp,
            tc.tile_pool(name="qslabp", bufs=1) as qslabp,
            tc.tile_pool(name="x1pool", bufs=1) as x1pool,
        ):
            kslab = kslabp.tile([128, NCQ, LH], bf16, tag="kslab")
            vslab = vslabp.tile([128, LH // 128, CQ], bf16, tag="vslab")
            qslab = qslabp.tile([128, NCQ, LCORE], bf16, tag="qslab")

            # small constants — all on the gpsimd queue BEFORE the
            # collectives occupy it
            sel_sb = singles.tile([128, 12], f32)
            nc.gpsimd.dma_start(sel_sb, selv[:, :])
            bq_sb = singles.tile([128, NCQ], f32)
            nc.gpsimd.dma_start(bq_sb, bq[:, :])
            bk_sb = singles.tile([128, NCQ], f32)
            nc.gpsimd.dma_start(bk_sb, bk[:, :])
            bv_sb = singles.tile([128, NCQ], f32)
            nc.gpsimd.dma_start(bv_sb, bv[:, :])
            bo_sb = singles.tile([128, NCO], f32)
            nc.gpsimd.dma_start(bo_sb, bo[:, :])
            am_sb = singles.tile([128, NBLK * NMC], f32)
            nc.gpsimd.dma_start(am_sb, amask[:, :])
            onesc_bf = singles.tile([128, 1], bf16)
            nc.vector.memset(onesc_bf, 1.0)
            onesr_bf = singles.tile([1, 128], bf16)
            nc.vector.memset(onesr_bf, 1.0)

            def load_x1_chunk(col0, ncols):
                """x1 chunk as 16 per-Ci tiles, issued in the order emit_v
                consumes them so the matmuls chase the DMAs."""
                tiles = []
                for Ci in range(NCI):
                    t = x1pool.tile([128, BL], bf16, tag=f"x1c{Ci}",
                                    name=f"x1c{Ci}", bufs=2 if Ci < 8 else 1)
                    nc.sync.dma_start(t[:, 0:ncols], x1r[:, Ci, ds(col0, ncols)])
                    tiles.append(t)
                return tiles

            # ---------------- Phase 1a: k and vT -> SBUF slabs -------------
            with (
                tc.tile_pool(name="wvp", bufs=1) as wvp,
                tc.tile_pool(name="wkp", bufs=1) as wkp,
                tc.tile_pool(name="psv", bufs=4, space="PSUM") as psv,
                tc.tile_pool(name="psk", bufs=3, space="PSUM") as psk,
            ):
                wv_t = []
                for ch in range(2):
                    wv_t.append(wvp.tile([128, NCI, BL], bf16, tag=f"wv{ch}",
                                         name=f"wv{ch}"))
                # interleave wv slice loads in consumption order (Ci-major)
                for Ci in range(NCI):
                    for ch in range(2):
                        nc.scalar.dma_start(wv_t[ch][:, Ci], wvB[:, ch, Ci])
                wk_t = []
                for ci in range(NCQ):
                    w = wkp.tile([128, NCI, 128], bf16, tag=f"wk{ci}",
                                 name=f"wk{ci}")
                    nc.scalar.dma_start(w, wkB[:, ci])
                    wk_t.append(w)

                def emit_v(x1t, col0, ncols):
                    for mo in range(ncols // 128):
                        row = (col0 + HALF) // 128 + mo
                        psA = psv.tile([128, BL], f32, tag="pv")
                        psB = psv.tile([128, BL], f32, tag="pv")
                        for Ci in range(NCI):
                            lhsT = x1t[Ci][:, ts(mo, 128)]
                            nc.tensor.matmul(
                                psA, lhsT=lhsT, rhs=wv_t[0][:, Ci],
                                start=(Ci == 0), stop=(Ci == NCI - 1),
                                skip_group_check=True)
                            nc.tensor.matmul(
                                psB, lhsT=lhsT, rhs=wv_t[1][:, Ci],
                                start=(Ci == 0), stop=(Ci == NCI - 1),
                                skip_group_check=True)
                        nc.vector.tensor_copy(vslab[:, row, 0:BL], psA)
                        nc.vector.tensor_copy(vslab[:, row, BL:2 * BL], psB)

                def emit_k(x1t, col0, ncols):
                    for ci in range(NCQ):
                        ps = psk.tile([128, ncols], f32, tag="pk")
                        for Ci in range(NCI):
                            nc.tensor.matmul(
                                ps,
                                lhsT=wk_t[ci][:, Ci],
                                rhs=x1t[Ci][:, 0:ncols],
                                start=(Ci == 0),
                                stop=(Ci == NCI - 1),
                            )
                        nc.scalar.add(
                            kslab[:, ci, ds(col0 + HALF, ncols)], ps,
                            bk_sb[:, ci:ci + 1])

                for li, (col0, ncols) in enumerate(LCHUNKS):
                    x1t = load_x1_chunk(col0, ncols)
                    emit_v(x1t, col0, ncols)
                    emit_k(x1t, col0, ncols)
                    if li == 0:
                        # left edge payload: own cols [0,256) = slab 256:512
                        for ci in range(NCQ):
                            nc.gpsimd.dma_start(
                                sk[:, 0, ci], kslab[:, ci, ds(HALF, HALF)])
                        for mo in range(2):
                            nc.gpsimd.dma_start(
                                sv[:, 2, mo], vslab[:, 2 + mo, :])
                    elif li == 1:
                        # right edge payload: own cols [1792,2048)
                        for ci in range(NCQ):
                            nc.gpsimd.dma_start(
                                sk[:, 1, ci],
                                kslab[:, ci, ds(LCORE, HALF)])
                        for mo in range(2):
                            nc.gpsimd.dma_start(
                                sv[:, 3, mo], vslab[:, 16 + mo, :])
                        for i in range(3):
                            nc.gpsimd.collective_compute(
                                "AllGather", mybir.AluOpType.bypass,
                                replica_groups=AGS[i],
                                ins=[send[:, :, :]],
                                outs=[recvs[i][:, :, :, :]])

            # wq reuses wv's address range (freed one k-chunk early; the
            # sync queue issues these as soon as that release fires);
            # wo later reuses wq's slots tag-by-tag.
            with tc.tile_pool(name="wqo", bufs=1) as wqo:
                wq_t = []
                for ci in range(NCQ):
                    w = wqo.tile([128, NCI, 128], bf16, tag=f"wz{ci}",
                                 name=f"wq{ci}")
                    nc.sync.dma_start(w, wqB[:, ci])
                    wq_t.append(w)

                # ---------------- halo blends (inside 1b) ----------------
                with tc.tile_pool(name="halop", bufs=1) as halop:

                    def blend(dst, cands, scol, shape):
                        acc = halop.tile(shape, bf16, tag="acc", name="acc")
                        for i, cnd in enumerate(cands):
                            cb = halop.tile(shape, bf16, tag="cand",
                                            name="cand", bufs=3)
                            nc.gpsimd.dma_start(cb, cnd)
                            if i == 0:
                                nc.vector.tensor_scalar_mul(
                                    acc, cb, sel_sb[:, scol:scol + 1])
                            else:
                                nc.vector.tensor_scalar_mul(
                                    cb, cb, sel_sb[:, scol + i:scol + i + 1])
                                nc.vector.tensor_add(acc, acc, cb)
                        nc.vector.tensor_copy(dst, acc)

                    # left halo source slot: kR(1)/vR(3); right: kL(0)/vL(2)
                    blend(kslab[:, :, ds(0, HALF)],
                          [rks[i][h, :, 1] for i in range(3) for h in range(2)],
                          0, [128, NCQ, HALF])
                    blend(kslab[:, :, ds(LH - HALF, HALF)],
                          [rks[i][h, :, 0] for i in range(3) for h in range(2)],
                          6, [128, NCQ, HALF])
                    blend(vslab[:, 0:2, :],
                          [rvs[i][h, :, 3] for i in range(3) for h in range(2)],
                          0, [128, 2, CQ])
                    blend(vslab[:, LH // 128 - 2:LH // 128, :],
                          [rvs[i][h, :, 2] for i in range(3) for h in range(2)],
                          6, [128, 2, CQ])

                    # ---------------- Phase 1b: q -> SBUF slab ----------------
                    with tc.tile_pool(name="psq", bufs=3, space="PSUM") as psq:
                        for c0 in range(0, LCORE, BL):
                            x1t = load_x1_chunk(c0, BL)
                            for ci in range(NCQ):
                                ps = psq.tile([128, BL], f32, tag="pq")
                                for Ci in range(NCI):
                                    nc.tensor.matmul(
                                        ps,
                                        lhsT=wq_t[ci][:, Ci],
                                        rhs=x1t[Ci],
                                        start=(Ci == 0),
                                        stop=(Ci == NCI - 1),
                                    )
                                nc.scalar.add(
                                    qslab[:, ci, ds(c0, BL)], ps,
                                    bq_sb[:, ci:ci + 1])

                # wo loads into wq's slots as q-projection releases them
                wo_t = []
                for ci in range(NCQ):
                    w = wqo.tile([128, C], bf16, tag=f"wz{ci}",
                                 name=f"wo{ci}")
                    nc.scalar.dma_start(w, wor[:, ci, :])
                    wo_t.append(w)

                # ---------------- Phase 2: attention + output proj --------
                with (
                    tc.tile_pool(name="ptp", bufs=8) as ptp,
                    tc.tile_pool(name="relup", bufs=1) as relup,
                    tc.tile_pool(name="small2", bufs=1) as small2,
                    tc.tile_pool(name="osbp", bufs=3) as osbp,
                    tc.tile_pool(name="tmpp", bufs=2) as tmpp,
                    tc.tile_pool(name="dacc", bufs=1) as daccp,
                    tc.tile_pool(name="psum2", bufs=3, space="PSUM") as psum2,
                    tc.tile_pool(name="psumav", bufs=3, space="PSUM") as psumav,
                    tc.tile_pool(name="psumd", bufs=2, space="PSUM") as psumd,
                ):
                    for b in BORDER:
                        # energy^T tiles (keys on partitions) + exp; denom
                        # partials accumulate on the DVE
                        acc = daccp.tile([128, BL], f32, tag="acc")
                        pts = []
                        for mc in range(NMC):
                            ps_st = psum2.tile([128, BL], f32, tag="st")
                            for ci in range(NCQ):
                                nc.tensor.matmul(
                                    ps_st,
                                    lhsT=kslab[:, ci, ds(b * BL + mc * 128, 128)],
                                    rhs=qslab[:, ci, ts(b, BL)],
                                    start=(ci == 0),
                                    stop=(ci == NCQ - 1),
                                )
                            pt = ptp.tile([128, BL], bf16, tag="pt")
                            col = b * NMC + mc
                            nc.scalar.activation(
                                pt, ps_st, AFT.Exp,
                                bias=am_sb[:, col:col + 1], scale=ESCALE)
                            pts.append(pt)
                            if mc == 0:
                                nc.vector.tensor_copy(acc, pt)
                            else:
                                nc.vector.tensor_add(acc, acc, pt)
                        accb = daccp.tile([128, BL], bf16, tag="accb")
                        nc.vector.tensor_copy(accb, acc)

                        # attention * V with the denominator's two tiny
                        # matmuls interleaved after the first two groups
                        relu_b = relup.tile([128, NCQ, BL], bf16, tag="relu")
                        av_ps = []
                        for ci in range(NCQ):
                            ps_av = psumav.tile([128, BL], f32, tag="av")
                            for mc in range(NMC):
                                nc.tensor.matmul(
                                    ps_av,
                                    lhsT=vslab[:, b * 4 + mc, ts(ci, 128)],
                                    rhs=pts[mc],
                                    start=(mc == 0),
                                    stop=(mc == NMC - 1),
                                )
                            av_ps.append(ps_av)
                            if ci == 0:
                                ps_den = psumd.tile([128, BL], f32, tag="den")
                                nc.tensor.matmul(
                                    ps_den[0:1, :], lhsT=onesc_bf, rhs=accb,
                                    start=True, stop=True)
                                recip = small2.tile([1, BL], f32, tag="recip")
                                nc.vector.reciprocal_approx_fast(
                                    recip, ps_den[0:1, :])
                                recip16 = small2.tile([1, BL], bf16,
                                                      tag="recip16")
                                nc.vector.tensor_copy(recip16, recip)
                            elif ci == 1:
                                ps_denb = psumd.tile([128, BL], f32, tag="den")
                                nc.tensor.matmul(
                                    ps_denb, lhsT=onesr_bf, rhs=recip16,
                                    start=True, stop=True)
                                recipb = small2.tile([128, BL], f32,
                                                     tag="recipb")
                                nc.vector.tensor_copy(recipb, ps_denb)
                        for ci in range(NCQ):
                            tmp = tmpp.tile([128, BL], f32, tag="tmp")
                            nc.vector.tensor_mul(tmp, av_ps[ci], recipb)
                            nc.scalar.activation(
                                relu_b[:, ci], tmp, AFT.Relu,
                                bias=bv_sb[:, ci:ci + 1], scale=1.0)

                        # output projection; result DMA rides the SP queue
                        for co in range(NCO):
                            ps_o = psum2.tile([128, BL], f32, tag="st")
                            for ci in range(NCQ):
                                nc.tensor.matmul(
                                    ps_o,
                                    lhsT=wo_t[ci][:, ts(co, 128)],
                                    rhs=relu_b[:, ci, :],
                                    start=(ci == 0),
                                    stop=(ci == NCQ - 1),
                                )
                            osb = osbp.tile([128, BL], f32, tag="osb")
                            nc.scalar.add(osb, ps_o, bo_sb[:, co:co + 1])
                            nc.sync.dma_start(outr[:, co, ts(b, BL)], osb)

    nc.finalize()
    return nc


def _part_major(v: np.ndarray) -> np.ndarray:
    """(n*128,) f32 vector -> (128, n) partition-major layout."""
    return np.ascontiguousarray(v.reshape(-1, 128).T).astype(np.float32)


def make_in_maps(x1, mask, wq, bq, wk, bk, wv, bv, wo, bo):
    X = np.asarray(x1[0], dtype=np.float32).astype(BF16)  # (C, L)

    wqT = np.ascontiguousarray(np.asarray(wq, np.float32).T).astype(BF16)
    wkT = np.ascontiguousarray(np.asarray(wk, np.float32).T).astype(BF16)
    wvT = np.ascontiguousarray(np.asarray(wv, np.float32).T).astype(BF16)
    woT = np.ascontiguousarray(np.asarray(wo, np.float32).T).astype(BF16)
    wkBd = np.ascontiguousarray(
        wkT.reshape(NCI, 128, NCQ, 128).transpose(1, 2, 0, 3))
    wqBd = np.ascontiguousarray(
        wqT.reshape(NCI, 128, NCQ, 128).transpose(1, 2, 0, 3))
    wvBd = np.ascontiguousarray(
        wvT.reshape(NCI, 128, 2, BL).transpose(1, 2, 0, 3))
    bqd = _part_major(np.asarray(bq, np.float32))
    bkd = _part_major(np.asarray(bk, np.float32))
    bvd = _part_major(np.asarray(bv, np.float32))
    bod = _part_major(np.asarray(bo, np.float32))

    # additive log-mask per global block: log(window_mask * padded_mask + 1e-9)
    pmpad = np.zeros(L + 2 * HALF, np.float32)
    pmpad[HALF:HALF + L] = np.asarray(mask, np.float32)[0, 0]
    wmcol = np.ones(WIN, np.float32)
    wmcol[-1] = 0.0
    nb_glob = L // BL
    fm = np.stack([wmcol * pmpad[bg * BL: bg * BL + WIN]
                   for bg in range(nb_glob)])  # (32, 1024)
    am_all = np.log(fm + 1e-9).astype(np.float32)

    axis_of = {1: 0, 2: 1, 4: 2}

    in_maps = [None] * NCORES
    for s in range(NCORES):       # shard index
        core = GRAY[s]
        x1sl = np.ascontiguousarray(X[:, s * LCORE:(s + 1) * LCORE])
        amc = am_all[s * NBLK:(s + 1) * NBLK]                # (4, 1024)
        amd = amc.reshape(NBLK, WIN // 128, 128).transpose(2, 0, 1)
        amd = np.ascontiguousarray(amd.reshape(128, NBLK * (WIN // 128)))
        # selectors: [left 6][right 6], entry (axis*2 + half)
        sel = np.zeros(12, np.float32)
        if s > 0:
            xl = GRAY[s - 1]
            ax = axis_of[xl ^ core]
            sel[ax * 2 + (1 if xl > core else 0)] = 1.0
        if s < NCORES - 1:
            xr = GRAY[s + 1]
            ax = axis_of[xr ^ core]
            sel[6 + ax * 2 + (1 if xr > core else 0)] = 1.0
        selc = np.ascontiguousarray(np.tile(sel, (128, 1)))
        in_maps[core] = {
            "x1s": x1sl, "wkB": wkBd, "wqB": wqBd, "wvB": wvBd, "woT": woT,
            "bq": bqd, "bk": bkd, "bv": bvd, "bo": bod, "amask": amd,
            "selv": selc,
        }
    return in_maps


_CACHED = {}


def kernel(**inputs) -> np.ndarray:
    x1 = np.asarray(inputs["x1"])
    mask = np.asarray(inputs["mask"])
    in_maps = make_in_maps(
        x1, mask,
        inputs["wq"], inputs["bq"], inputs["wk"], inputs["bk"],
        inputs["wv"], inputs["bv"], inputs["wo"], inputs["bo"])

    from concourse.bass_utils import run_bass_kernel_spmd

    if "nc" not in _CACHED:
        _CACHED["nc"] = build_kernel()
    nc = _CACHED["nc"]

    res = run_bass_kernel_spmd(nc, in_maps, core_ids=list(range(NCORES)))
    # core GRAY[s] holds shard s
    outs = [np.asarray(res.results[GRAY[s]]["out"]) for s in range(NCORES)]
    full = np.concatenate(outs, axis=1)[None]          # (1, C, L)
    full = full * np.asarray(mask, np.float32)[:, 0:1, :]
    return np.ascontiguousarray(full.astype(np.float32))


if __name__ == "__main__":
    nc = build_kernel()
    print("built ok")


# revision 16
# speedup vs baseline: 1.2056x; 1.1596x over previous
"""Trainium2 Bass kernel for the sliding-window (sparse block) attention layer.

Problem shape: B=1, C=2048, L=16384, projected c=1024, block bl=512, nb=32
blocks, window 2*bl=1024 with halo bl//2=256.

Sharding: sequence-parallel over the nb block dimension; each of the 8 cores
owns 4 consecutive blocks (2048 columns).  The k/v halo (bl//2 = 256 columns
each side) is RECOMPUTED from an overlapping x1 slab (2560 columns per core,
zero-padded at the global sequence ends) instead of being exchanged: the
pairwise AllGather halo exchange used in earlier versions measurably
throttled the whole chip while its ncfw/SDMA machinery ran (~2600 matmuls
slowed from 216ns to 263ns), costing far more than the ~55us of duplicated
projection work.

All of k, vT and q live in SBUF across phases (kslab 40KB + vslab 40KB +
qslab 32KB per partition); no DRAM staging, no collectives:

  Phase 1a (single x1 pass over the 2560-col slab, five 512-col chunks):
    vT = (wv@x1)^T computed directly in transposed layout (interleaved
    dual-psum groups sharing each x1 LDWEIGHTS) and k = wk@x1+bk, both
    evacuated straight into the SBUF slabs.  x1 arrives in 3 region tiles /
    5 coalesced DMAs per chunk (the HWDGE sequencer costs ~600ns per
    dma_start); lo/mid/hi double-buffer, the last two Ci single-buffer and
    chase.  v runs before k in every chunk so the wv pool frees one chunk
    early and wq (reusing wv's address range) prefetches during the last
    k chunk.
  Phase 1b (second x1 pass over the own 2048 columns): q = wq@x1+bq.
  Phase 2 (per block): S^T = k^T q per 128-key chunk; P~^T =
    exp(S^T/sqrt(c)+logmask) via one ACT op (per-key log-mask rides the
    ACT bias); softmax denominator via DVE running sum + a ones-matmul +
    reciprocal + a broadcast ones-matmul, interleaved between the first AV
    groups so the PE never waits; out = vT^T P~^T, *recip in-place in PSUM,
    relu(+bv); final = wo^T relu + bo, DMA'd out alternating SP/ACT queues.
    wo loads into wq's slots (same pool tags) as the last q-projection
    groups release them.

The reference's softmax(energy + log(mask+1e-9)) is reproduced exactly by
the additive log-mask; the post-softmax *mask multiply only zeroes
~1e-9-weight entries and is skipped (the final `out * mask` is host-side).
The zero-padded slab ends reproduce the reference's k/v zero padding (the
k bias lands on dead, fully-masked columns there; biases are zero in this
model regardless).
"""

import os
import sys

import numpy as np

for _p in ("/root/.axon_site", "/root/.axon_site/_ro/trn_rl_repo", "/opt/trn_rl_repo"):
    if os.path.isdir(_p) and _p not in sys.path:
        sys.path.append(_p)

import ml_dtypes

import concourse.bass as bass
import concourse.mybir as mybir
import concourse.tile as tile
from concourse import bacc
from concourse.bass import ds, ts

BF16 = ml_dtypes.bfloat16

# Model dims (hardcoded per problem spec)
C = 2048          # input channels
CQ = 1024         # projected channels
L = 16384         # sequence length
BL = 512          # block length
HALF = 256        # halo = BL // 2
NCORES = 8
LCORE = L // NCORES          # 2048 owned columns per core
LH = LCORE + 2 * HALF        # 2560 slab columns incl halo
NBLK = LCORE // BL           # 4 blocks per core
WIN = 2 * BL                 # 1024 attention window
ESCALE = 1.0 / float(np.sqrt(CQ))  # 1/32

NCI = C // 128    # 16 contraction chunks over C
NCQ = CQ // 128   # 8 chunks over projected c
NCO = C // 128    # 16 chunks over output channels
NMC = WIN // 128  # 8 key chunks per window

GRAY = list(range(NCORES))   # shard s on core s (kept for test.py)
# phase-2 block order
BORDER = [0, 1, 2, 3]


def build_kernel() -> bass.Bass:
    nc = bacc.Bacc("TRN2", target_bir_lowering=False, num_devices=NCORES)
    dt = mybir.dt
    f32, bf16 = dt.float32, dt.bfloat16
    AFT = mybir.ActivationFunctionType

    x1s = nc.dram_tensor("x1s", [C, LH], bf16, kind="ExternalInput")
    wkB = nc.dram_tensor("wkB", [128, NCQ, NCI, 128], bf16, kind="ExternalInput")
    wqB = nc.dram_tensor("wqB", [128, NCQ, NCI, 128], bf16, kind="ExternalInput")
    wvB = nc.dram_tensor("wvB", [128, 2, NCI, BL], bf16, kind="ExternalInput")
    woT = nc.dram_tensor("woT", [CQ, C], bf16, kind="ExternalInput")
    bq = nc.dram_tensor("bq", [128, CQ // 128], f32, kind="ExternalInput")
    bk = nc.dram_tensor("bk", [128, CQ // 128], f32, kind="ExternalInput")
    bv = nc.dram_tensor("bv", [128, CQ // 128], f32, kind="ExternalInput")
    bo = nc.dram_tensor("bo", [128, C // 128], f32, kind="ExternalInput")
    amask = nc.dram_tensor("amask", [128, NBLK * (WIN // 128)], f32,
                           kind="ExternalInput")
    out = nc.dram_tensor("out", [C, LCORE], f32, kind="ExternalOutput")

    x1r = x1s.rearrange("(ci p) l -> p ci l", p=128)    # (128, 16, 2560)
    wor = woT.rearrange("(ci p) co -> p ci co", p=128)  # (128, 8, 2048)
    outr = out.rearrange("(co p) l -> p co l", p=128)   # (128, 16, 2048)

    with tile.TileContext(nc) as tc:
        with (
            tc.tile_pool(name="singles", bufs=1) as singles,
            tc.tile_pool(name="kslabp", bufs=1) as kslabp,
            tc.tile_pool(name="vslabp", bufs=1) as vslabp,
            tc.tile_pool(name="qslabp", bufs=1) as qslabp,
            tc.tile_pool(name="x1pool", bufs=1) as x1pool,
        ):
            kslab = kslabp.tile([128, NCQ, LH], bf16, tag="kslab")
            vslab = vslabp.tile([128, LH // 128, CQ], bf16, tag="vslab")
            qslab = qslabp.tile([128, NCQ, LCORE], bf16, tag="qslab")

            bq_sb = singles.tile([128, NCQ], f32)
            nc.gpsimd.dma_start(bq_sb, bq[:, :])
            bk_sb = singles.tile([128, NCQ], f32)
            nc.gpsimd.dma_start(bk_sb, bk[:, :])
            bv_sb = singles.tile([128, NCQ], f32)
            nc.gpsimd.dma_start(bv_sb, bv[:, :])
            bo_sb = singles.tile([128, NCO], f32)
            nc.gpsimd.dma_start(bo_sb, bo[:, :])
            am_sb = singles.tile([128, NBLK * NMC], f32)
            nc.gpsimd.dma_start(am_sb, amask[:, :])
            onesc_bf = singles.tile([128, 1], bf16)
            nc.vector.memset(onesc_bf, 1.0)
            onesr_bf = singles.tile([1, 128], bf16)
            nc.vector.memset(onesr_bf, 1.0)

            def load_x1_chunk(col0, ncols):
                """x1 chunk in 4 region tiles / 5 coalesced DMAs, issued in
                the order emit_v consumes them so the matmuls chase the
                DMAs.  col0 is a slab coordinate."""
                lo = x1pool.tile([128, 8, BL], bf16, tag="x1lo", name="x1lo",
                                 bufs=2)
                mid = x1pool.tile([128, 4, BL], bf16, tag="x1mid", name="x1mid",
                                  bufs=2)
                hi = x1pool.tile([128, 2, BL], bf16, tag="x1hi", name="x1hi",
                                 bufs=2)
                hi2 = x1pool.tile([128, 2, BL], bf16, tag="x1hi2", name="x1hi2",
                                  bufs=1)
                nc.sync.dma_start(lo[:, 0:4, 0:ncols],
                                  x1r[:, 0:4, ds(col0, ncols)])
                nc.sync.dma_start(lo[:, 4:8, 0:ncols],
                                  x1r[:, 4:8, ds(col0, ncols)])
                nc.sync.dma_start(mid[:, :, 0:ncols],
                                  x1r[:, 8:12, ds(col0, ncols)])
                nc.sync.dma_start(hi[:, :, 0:ncols],
                                  x1r[:, 12:14, ds(col0, ncols)])
                nc.sync.dma_start(hi2[:, :, 0:ncols],
                                  x1r[:, 14:16, ds(col0, ncols)])

                def sl(Ci):
                    if Ci < 8:
                        return lo[:, Ci]
                    if Ci < 12:
                        return mid[:, Ci - 8]
                    if Ci < 14:
                        return hi[:, Ci - 12]
                    return hi2[:, Ci - 14]
                return [sl(Ci) for Ci in range(NCI)]

            # ---------------- Phase 1a: k and vT -> SBUF slabs -------------
            with (
                tc.tile_pool(name="wvp", bufs=1) as wvp,
                tc.tile_pool(name="wkp", bufs=1) as wkp,
                tc.tile_pool(name="psv", bufs=4, space="PSUM") as psv,
                tc.tile_pool(name="psk", bufs=3, space="PSUM") as psk,
            ):
                # wv on the scalar queue in 8 quarter-loads interleaved in
                # consumption order (Ci-major); wk rides the sync queue
                # after the first x1 chunk.
                wv_t = []
                for ch in range(2):
                    wv_t.append(wvp.tile([128, NCI, BL], bf16, tag=f"wv{ch}",
                                         name=f"wv{ch}"))
                for Ci0 in range(0, NCI, 4):
                    for ch in range(2):
                        nc.scalar.dma_start(wv_t[ch][:, Ci0:Ci0 + 4],
                                            wvB[:, ch, Ci0:Ci0 + 4])
                wk_t = []
                for ci in range(NCQ):
                    wk_t.append(wkp.tile([128, NCI, 128], bf16, tag=f"wk{ci}",
                                         name=f"wk{ci}"))

                def emit_v(x1t, col0, ncols):
                    for mo in range(ncols // 128):
                        row = col0 // 128 + mo
                        psA = psv.tile([128, BL], f32, tag="pv")
                        psB = psv.tile([128, BL], f32, tag="pv")
                        for Ci in range(NCI):
                            lhsT = x1t[Ci][:, ts(mo, 128)]
                            nc.tensor.matmul(
                                psA, lhsT=lhsT, rhs=wv_t[0][:, Ci],
                                start=(Ci == 0), stop=(Ci == NCI - 1),
                                skip_group_check=True)
                            nc.tensor.matmul(
                                psB, lhsT=lhsT, rhs=wv_t[1][:, Ci],
                                start=(Ci == 0), stop=(Ci == NCI - 1),
                                skip_group_check=True)
                        nc.vector.tensor_copy(vslab[:, row, 0:BL], psA)
                        nc.vector.tensor_copy(vslab[:, row, BL:2 * BL], psB)

                def emit_k(x1t, col0, ncols):
                    for ci in range(NCQ):
                        ps = psk.tile([128, ncols], f32, tag="pk")
                        for Ci in range(NCI):
                            nc.tensor.matmul(
                                ps,
                                lhsT=wk_t[ci][:, Ci],
                                rhs=x1t[Ci][:, 0:ncols],
                                start=(Ci == 0),
                                stop=(Ci == NCI - 1),
                            )
                        nc.scalar.add(
                            kslab[:, ci, ds(col0, ncols)], ps,
                            bk_sb[:, ci:ci + 1])

                for li in range(5):
                    col0 = li * BL
                    x1t = load_x1_chunk(col0, BL)
                    if li == 0:
                        for ci in range(NCQ):
                            nc.sync.dma_start(wk_t[ci], wkB[:, ci])
                    emit_v(x1t, col0, BL)
                    emit_k(x1t, col0, BL)

            # wq reuses wv's address range (freed one k-chunk early; the
            # sync queue issues these as soon as that release fires);
            # wo later reuses wq's slots tag-by-tag.
            with tc.tile_pool(name="wqo", bufs=1) as wqo:
                wq_t = []
                for ci in range(NCQ):
                    w = wqo.tile([128, NCI, 128], bf16, tag=f"wz{ci}",
                                 name=f"wq{ci}")
                    nc.sync.dma_start(w, wqB[:, ci])
                    wq_t.append(w)

                # ---------------- Phase 1b: q -> SBUF slab ----------------
                with tc.tile_pool(name="psq", bufs=3, space="PSUM") as psq:
                    for c0 in range(0, LCORE, BL):
                        x1t = load_x1_chunk(HALF + c0, BL)
                        for ci in range(NCQ):
                            ps = psq.tile([128, BL], f32, tag="pq")
                            for Ci in range(NCI):
                                nc.tensor.matmul(
                                    ps,
                                    lhsT=wq_t[ci][:, Ci],
                                    rhs=x1t[Ci],
                                    start=(Ci == 0),
                                    stop=(Ci == NCI - 1),
                                )
                            nc.scalar.add(
                                qslab[:, ci, ds(c0, BL)], ps,
                                bq_sb[:, ci:ci + 1])

                # wo loads into wq's slots as q-projection releases them
                wo_t = []
                for ci in range(NCQ):
                    w = wqo.tile([128, C], bf16, tag=f"wz{ci}",
                                 name=f"wo{ci}")
                    nc.scalar.dma_start(w, wor[:, ci, :])
                    wo_t.append(w)

                # ---------------- Phase 2: attention + output proj --------
                with (
                    tc.tile_pool(name="ptp", bufs=8) as ptp,
                    tc.tile_pool(name="relup", bufs=1) as relup,
                    tc.tile_pool(name="small2", bufs=1) as small2,
                    tc.tile_pool(name="osbp", bufs=3) as osbp,
                    tc.tile_pool(name="dacc", bufs=1) as daccp,
                    tc.tile_pool(name="psum2", bufs=3, space="PSUM") as psum2,
                    tc.tile_pool(name="psumav", bufs=3, space="PSUM") as psumav,
                    tc.tile_pool(name="psumd", bufs=2, space="PSUM") as psumd,
                ):
                    for b in BORDER:
                        # energy^T tiles (keys on partitions) + exp; denom
                        # partials accumulate on the DVE
                        acc = daccp.tile([128, BL], f32, tag="acc")
                        pts = []
                        for mc in range(NMC):
                            ps_st = psum2.tile([128, BL], f32, tag="st")
                            for ci in range(NCQ):
                                nc.tensor.matmul(
                                    ps_st,
                                    lhsT=kslab[:, ci, ds(b * BL + mc * 128, 128)],
                                    rhs=qslab[:, ci, ts(b, BL)],
                                    start=(ci == 0),
                                    stop=(ci == NCQ - 1),
                                )
                            pt = ptp.tile([128, BL], bf16, tag="pt")
                            col = b * NMC + mc
                            nc.scalar.activation(
                                pt, ps_st, AFT.Exp,
                                bias=am_sb[:, col:col + 1], scale=ESCALE)
                            pts.append(pt)
                            if mc == 0:
                                nc.vector.tensor_copy(acc, pt)
                            else:
                                nc.vector.tensor_add(acc, acc, pt)
                        accb = daccp.tile([128, BL], bf16, tag="accb")
                        nc.vector.tensor_copy(accb, acc)

                        # attention * V with the denominator's two tiny
                        # matmuls interleaved after the first two groups
                        relu_b = relup.tile([128, NCQ, BL], bf16, tag="relu")
                        av_ps = []
                        for ci in range(NCQ):
                            ps_av = psumav.tile([128, BL], f32, tag="av")
                            for mc in range(NMC):
                                nc.tensor.matmul(
                                    ps_av,
                                    lhsT=vslab[:, b * 4 + mc, ts(ci, 128)],
                                    rhs=pts[mc],
                                    start=(mc == 0),
                                    stop=(mc == NMC - 1),
                                )
                            av_ps.append(ps_av)
                            if ci == 0:
                                ps_den = psumd.tile([128, BL], f32, tag="den")
                                nc.tensor.matmul(
                                    ps_den[0:1, :], lhsT=onesc_bf, rhs=accb,
                                    start=True, stop=True)
                                recip = small2.tile([1, BL], f32, tag="recip")
                                nc.vector.reciprocal_approx_fast(
                                    recip, ps_den[0:1, :])
                                recip16 = small2.tile([1, BL], bf16,
                                                      tag="recip16")
                                nc.vector.tensor_copy(recip16, recip)
                            elif ci == 1:
                                ps_denb = psumd.tile([128, BL], f32, tag="den")
                                nc.tensor.matmul(
                                    ps_denb, lhsT=onesr_bf, rhs=recip16,
                                    start=True, stop=True)
                                recipb = small2.tile([128, BL], f32,
                                                     tag="recipb")
                                nc.vector.tensor_copy(recipb, ps_denb)
                        for ci in range(NCQ):
                            nc.vector.tensor_mul(av_ps[ci], av_ps[ci], recipb)
                            nc.scalar.activation(
                                relu_b[:, ci], av_ps[ci], AFT.Relu,
                                bias=bv_sb[:, ci:ci + 1], scale=1.0)

                        # output projection; result DMAs alternate queues
                        for co in range(NCO):
                            ps_o = psum2.tile([128, BL], f32, tag="st")
                            for ci in range(NCQ):
                                nc.tensor.matmul(
                                    ps_o,
                                    lhsT=wo_t[ci][:, ts(co, 128)],
                                    rhs=relu_b[:, ci, :],
                                    start=(ci == 0),
                                    stop=(ci == NCQ - 1),
                                )
                            osb = osbp.tile([128, BL], f32, tag="osb")
                            nc.scalar.add(osb, ps_o, bo_sb[:, co:co + 1])
                            eng = nc.sync if co % 2 else nc.scalar
                            eng.dma_start(outr[:, co, ts(b, BL)], osb)

    nc.finalize()
    return nc


def _part_major(v: np.ndarray) -> np.ndarray:
    """(n*128,) f32 vector -> (128, n) partition-major layout."""
    return np.ascontiguousarray(v.reshape(-1, 128).T).astype(np.float32)


def make_in_maps(x1, mask, wq, bq, wk, bk, wv, bv, wo, bo):
    X = np.asarray(x1[0], dtype=np.float32).astype(BF16)  # (C, L)
    Xp = np.zeros((C, L + 2 * HALF), BF16)
    Xp[:, HALF:HALF + L] = X

    wqT = np.ascontiguousarray(np.asarray(wq, np.float32).T).astype(BF16)
    wkT = np.ascontiguousarray(np.asarray(wk, np.float32).T).astype(BF16)
    wvT = np.ascontiguousarray(np.asarray(wv, np.float32).T).astype(BF16)
    woT = np.ascontiguousarray(np.asarray(wo, np.float32).T).astype(BF16)
    wkBd = np.ascontiguousarray(
        wkT.reshape(NCI, 128, NCQ, 128).transpose(1, 2, 0, 3))
    wqBd = np.ascontiguousarray(
        wqT.reshape(NCI, 128, NCQ, 128).transpose(1, 2, 0, 3))
    wvBd = np.ascontiguousarray(
        wvT.reshape(NCI, 128, 2, BL).transpose(1, 2, 0, 3))
    bqd = _part_major(np.asarray(bq, np.float32))
    bkd = _part_major(np.asarray(bk, np.float32))
    bvd = _part_major(np.asarray(bv, np.float32))
    bod = _part_major(np.asarray(bo, np.float32))

    # additive log-mask per global block: log(window_mask * padded_mask + 1e-9)
    pmpad = np.zeros(L + 2 * HALF, np.float32)
    pmpad[HALF:HALF + L] = np.asarray(mask, np.float32)[0, 0]
    wmcol = np.ones(WIN, np.float32)
    wmcol[-1] = 0.0
    nb_glob = L // BL
    fm = np.stack([wmcol * pmpad[bg * BL: bg * BL + WIN]
                   for bg in range(nb_glob)])  # (32, 1024)
    am_all = np.log(fm + 1e-9).astype(np.float32)

    in_maps = [None] * NCORES
    for s in range(NCORES):       # shard index == core index
        x1sl = np.ascontiguousarray(Xp[:, s * LCORE:s * LCORE + LH])
        amc = am_all[s * NBLK:(s + 1) * NBLK]                # (4, 1024)
        amd = amc.reshape(NBLK, WIN // 128, 128).transpose(2, 0, 1)
        amd = np.ascontiguousarray(amd.reshape(128, NBLK * (WIN // 128)))
        in_maps[s] = {
            "x1s": x1sl, "wkB": wkBd, "wqB": wqBd, "wvB": wvBd, "woT": woT,
            "bq": bqd, "bk": bkd, "bv": bvd, "bo": bod, "amask": amd,
        }
    return in_maps


_CACHED = {}


def kernel(**inputs) -> np.ndarray:
    x1 = np.asarray(inputs["x1"])
    mask = np.asarray(inputs["mask"])
    in_maps = make_in_maps(
        x1, mask,
        inputs["wq"], inputs["bq"], inputs["wk"], inputs["bk"],
        inputs["wv"], inputs["bv"], inputs["wo"], inputs["bo"])

    from concourse.bass_utils import run_bass_kernel_spmd

    if "nc" not in _CACHED:
        _CACHED["nc"] = build_kernel()
    nc = _CACHED["nc"]

    res = run_bass_kernel_spmd(nc, in_maps, core_ids=list(range(NCORES)))
    outs = [np.asarray(res.results[s]["out"]) for s in range(NCORES)]
    full = np.concatenate(outs, axis=1)[None]          # (1, C, L)
    full = full * np.asarray(mask, np.float32)[:, 0:1, :]
    return np.ascontiguousarray(full.astype(np.float32))


if __name__ == "__main__":
    nc = build_kernel()
    print("built ok")
